# revision 16
# baseline (speedup 1.0000x reference)
"""MoE FFN (8 experts, top-2) on 8 TRN2 NeuronCores — expert-parallel.

Strategy:
  - Host (inside kernel()): compute gate logits (fp64), top-2 selection +
    softmax weights exactly as the reference; gather each expert's tokens
    into a padded transposed buffer (bf16).
  - Device (SPMD, one expert per core): hT = relu(W1^T @ x + b1) then
    yT = W2^T @ hT + b2, bf16 matmuls with fp32 PSUM accumulation.
    All inputs are host-packed so every DMA reads >=6KB contiguous rows.
  - Host: out[token] += gate_weight * yT[:, col].T  (scatter-add combine).

Shapes (hardcoded from the problem):
  x: [4, 1024, 1024] f32, Wg: [1024, 8], bg: [8],
  W1: [8, 1024, 4096], b1: [8, 4096], W2: [8, 4096, 1024], b2: [8, 1024]
"""

import math

import ml_dtypes
import numpy as np

MODEL_DIM = 1024
DIM_FF = 4096
NUM_EXPERTS = 8
TOP_K = 2
N_CORES = 8

BF16 = ml_dtypes.bfloat16

_NC_CACHE: dict[int, object] = {}


def _ttile_split(C):
    n_tt = math.ceil(C / 512)
    ncols = [C // n_tt + (1 if i < C % n_tt else 0) for i in range(n_tt)]
    col0s = [sum(ncols[:i]) for i in range(n_tt)]
    return n_tt, ncols, col0s


def _build_moe_nc(C: int):
    """Build + compile the per-core Bass program for token capacity C.

    Inputs (per core, all host-packed for contiguous DMA):
      xs   [128, 8*C]  bf16 — token block tt at cols 8*col0[tt], dk-major inside
      w1   [128, 32768] bf16 — block (c,h) at col (c*2+h)*4096; inside, dk=4h+j
                               occupies cols j*1024..; covers W1[dk-rows, c-cols]
      w2   [128, 32768] bf16 — block g at col g*4096; fk=4g+j at j*1024..
      bias [128, 40]   f32  — cols 0:32 = b1 f-blocks, 32:40 = b2 d-blocks
    Output: y [1024, C] bf16 (= W2^T @ relu(W1^T @ xs + b1) + b2, transposed).
    """
    import concourse.mybir as mybir
    import concourse.tile as tile
    from concourse import bacc

    d, f = MODEL_DIM, DIM_FF
    DKS = d // 128   # 8
    FKS = f // 128   # 32
    n_tt, ncols, col0s = _ttile_split(C)

    # head = xs-tt0 + fm0..1 lhsT dup + bias(f32 bits as bf16 cols), ONE DMA
    HEAD_XS = 8 * ncols[0]
    HEAD_COLS = HEAD_XS + 2048 + 80

    nc = bacc.Bacc("TRN2", target_bir_lowering=False)
    head_d = nc.dram_tensor("head", [128, HEAD_COLS], mybir.dt.bfloat16,
                            kind="ExternalInput")
    xs_d = nc.dram_tensor("xs", [128, 8 * C], mybir.dt.bfloat16, kind="ExternalInput")
    w1_d = nc.dram_tensor("w1", [128, 32768], mybir.dt.bfloat16, kind="ExternalInput")
    w2_d = nc.dram_tensor("w2", [128, 32768], mybir.dt.bfloat16, kind="ExternalInput")
    y_d = nc.dram_tensor("y", [d, C], mybir.dt.bfloat16, kind="ExternalOutput")

    with tile.TileContext(nc) as tc:
        with (
            tc.tile_pool(name="pers", bufs=1) as pers,
            tc.tile_pool(name="xsp", bufs=2) as xsp,
            tc.tile_pool(name="hp", bufs=1) as hp,
            tc.tile_pool(name="yp", bufs=2) as yp,
            tc.tile_pool(name="psp", bufs=8, space="PSUM") as psp,
        ):
            headt = pers.tile([128, HEAD_COLS], mybir.dt.bfloat16,
                              tag="head", name="headt")
            nc.sync.dma_start(headt, head_d[:, :])

            # HAM warmup: dummy matmuls on a memset tile keep the PE busy
            # during the initial DMA wait so real matmuls start at 2.4 GHz.
            warm_sb = pers.tile([128, 512], mybir.dt.bfloat16,
                                tag="warm", name="warm_sb")
            nc.vector.memset(warm_sb, 0)
            warm_ps = psp.tile([128, 512], mybir.dt.float32, tag="ps",
                               name="warm_ps")
            N_WARM = 36
            for i in range(N_WARM):
                nc.tensor.matmul(
                    warm_ps, lhsT=warm_sb[:, :128], rhs=warm_sb,
                    start=(i == 0), stop=(i == N_WARM - 1),
                )

            xst = {0: headt[:, :HEAD_XS]}
            w1ht = headt[:, HEAD_XS:HEAD_XS + 2048]
            biast = headt[:, HEAD_XS + 2048:HEAD_XS + 2128].bitcast(
                mybir.dt.float32)

            def issue_xs(tt):
                ncol = ncols[tt]
                t = xsp.tile([128, 8 * ncol], mybir.dt.bfloat16,
                             tag="xs", name=f"xst_{tt}")
                nc.sync.dma_start(
                    t, xs_d[:, 8 * col0s[tt]:8 * col0s[tt] + 8 * ncol]
                )
                xst[tt] = t

            # w1 blocks (c, h): issued c-major to match phase A's fm-order reads
            w1t = {}
            for c in range(4):
                for h in range(2):
                    t = pers.tile([128, 4096], mybir.dt.bfloat16,
                                  tag=f"w1_{c}_{h}", name=f"w1t_{c}_{h}")
                    off = (c * 2 + h) * 4096
                    nc.sync.dma_start(t, w1_d[:, off:off + 4096])
                    w1t[c, h] = t
            if n_tt > 1:
                issue_xs(1)
            w2t = {}
            for g in range(8):
                t = pers.tile([128, 4096], mybir.dt.bfloat16,
                              tag=f"w2_{g}", name=f"w2t_{g}")
                nc.sync.dma_start(t, w2_d[:, g * 4096:(g + 1) * 4096])
                w2t[g] = t
            if n_tt > 2:
                issue_xs(2)

            for tt in range(n_tt):
                ncol, col0 = ncols[tt], col0s[tt]
                nxt = tt + 2
                if 3 <= nxt < n_tt:
                    issue_xs(nxt)

                # Phase A: hT[fm-block, cols] = relu(W1^T @ x + b1)
                hts = []
                for fm in range(FKS):
                    ps = psp.tile([128, ncol], mybir.dt.float32, tag="ps",
                                  name=f"ps1_{fm}_{tt}")
                    for dk in range(DKS):
                        if tt == 0 and fm < 2:
                            lhsT = w1ht[:, fm * 1024 + dk * 128:
                                        fm * 1024 + (dk + 1) * 128]
                        else:
                            lo = (dk % 4) * 1024 + (fm % 8) * 128
                            lhsT = w1t[fm // 8, dk // 4][:, lo:lo + 128]
                        nc.tensor.matmul(
                            ps,
                            lhsT=lhsT,
                            rhs=xst[tt][:, dk * ncol:(dk + 1) * ncol],
                            start=(dk == 0),
                            stop=(dk == DKS - 1),
                        )
                    ht = hp.tile([128, ncol], mybir.dt.bfloat16,
                                 tag=f"h_{fm}", name=f"ht_{fm}_{tt}")
                    # ht = max(ps + b1, 0) in one DVE op
                    nc.vector.tensor_scalar(
                        out=ht, in0=ps,
                        scalar1=biast[:, fm:fm + 1], scalar2=0.0,
                        op0=mybir.AluOpType.add, op1=mybir.AluOpType.max,
                    )
                    hts.append(ht)

                # Phase B: yT[dm-block, cols] = W2^T @ hT + b2
                for dm in range(DKS):
                    ps = psp.tile([128, ncol], mybir.dt.float32, tag="ps",
                                  name=f"ps2_{dm}_{tt}")
                    for fk in range(FKS):
                        lo = (fk % 4) * 1024 + dm * 128
                        nc.tensor.matmul(
                            ps,
                            lhsT=w2t[fk // 4][:, lo:lo + 128],
                            rhs=hts[fk],
                            start=(fk == 0),
                            stop=(fk == FKS - 1),
                        )
                    yo = yp.tile([128, ncol], mybir.dt.bfloat16, tag="yo",
                                 name=f"yo_{dm}_{tt}")
                    nc.vector.tensor_scalar_add(yo, ps, biast[:, 32 + dm:33 + dm])
                    nc.sync.dma_start(
                        y_d[dm * 128:(dm + 1) * 128, col0:col0 + ncol], yo
                    )

    nc.compile()
    return nc


def _pack_w1(W1e):
    """W1e [1024, 4096] f32 -> [128, 32768] bf16, block (c,h) layout."""
    W = W1e.astype(BF16)
    blocks = []
    for c in range(4):
        for h in range(2):
            blocks.append(np.concatenate(
                [W[(4 * h + j) * 128:(4 * h + j + 1) * 128,
                   c * 1024:(c + 1) * 1024] for j in range(4)], axis=1))
    return np.ascontiguousarray(np.concatenate(blocks, axis=1))


def _pack_w2(W2e):
    """W2e [4096, 1024] f32 -> [128, 32768] bf16, block g layout."""
    W = W2e.astype(BF16)
    blocks = []
    for g in range(8):
        blocks.append(np.concatenate(
            [W[(4 * g + j) * 128:(4 * g + j + 1) * 128, :] for j in range(4)],
            axis=1))
    return np.ascontiguousarray(np.concatenate(blocks, axis=1))


def _pack_w1h(W1e):
    """fm=0..1 lhsT blocks: [128, 2048] bf16, col fm*1024 + dk*128 -> W1 block."""
    W = W1e.astype(BF16)
    out = np.empty((128, 2048), dtype=BF16)
    for fm in range(2):
        for dk in range(8):
            out[:, fm * 1024 + dk * 128:fm * 1024 + (dk + 1) * 128] = \
                W[dk * 128:(dk + 1) * 128, fm * 128:(fm + 1) * 128]
    return np.ascontiguousarray(out)


def _pack_bias(b1e, b2e):
    bias = np.zeros((128, 40), np.float32)
    bias[:, :32] = b1e.astype(np.float32).reshape(32, 128).T
    bias[:, 32:40] = b2e.astype(np.float32).reshape(8, 128).T
    return np.ascontiguousarray(bias)


def _pack_xs(xT_bf16, C):
    """xT [1024, C] bf16 -> [128, 8*C] packed tt-major, dk inside."""
    _, ncols, col0s = _ttile_split(C)
    out = np.empty((128, 8 * C), dtype=BF16)
    for tt, (ncol, col0) in enumerate(zip(ncols, col0s)):
        for dk in range(8):
            out[:, 8 * col0 + dk * ncol: 8 * col0 + (dk + 1) * ncol] = \
                xT_bf16[dk * 128:(dk + 1) * 128, col0:col0 + ncol]
    return np.ascontiguousarray(out)


def _route_host(x, Wg, bg):
    """Reference-exact gate: fp64 logits, top-2 (ties -> lower index), softmax."""
    T = x.shape[0]
    logits = x.astype(np.float64) @ Wg.astype(np.float64) + bg.astype(np.float64)
    order = np.argsort(-logits, axis=1, kind="stable")[:, :TOP_K]  # [T, 2]
    vals = np.take_along_axis(logits, order, axis=1)
    vmax = vals.max(axis=1, keepdims=True)
    ev = np.exp(vals - vmax)
    w = (ev / ev.sum(axis=1, keepdims=True)).astype(np.float32)  # [T, 2]
    tok = np.repeat(np.arange(T), TOP_K)
    exp = order.ravel()
    wgt = w.ravel()
    tok_e, w_e = [], []
    for e in range(NUM_EXPERTS):
        m = exp == e
        tok_e.append(tok[m])
        w_e.append(wgt[m])
    return tok_e, w_e


def _make_in_maps(xt, W1, b1, W2, b2, tok_e, counts, C):
    _, ncols, _ = _ttile_split(C)
    in_maps = []
    for e in range(NUM_EXPERTS):
        xT = np.zeros((MODEL_DIM, C), dtype=BF16)
        cnt = counts[e]
        if cnt:
            xT[:, :cnt] = xt[tok_e[e]].astype(BF16).T
        xs_packed = _pack_xs(xT, C)
        head = np.ascontiguousarray(np.concatenate([
            xs_packed[:, :8 * ncols[0]],
            _pack_w1h(W1[e]),
            _pack_bias(b1[e], b2[e]).view(BF16),
        ], axis=1))
        in_maps.append({
            "head": head,
            "xs": xs_packed,
            "w1": _pack_w1(W1[e]),
            "w2": _pack_w2(W2[e]),
        })
    return in_maps


def kernel(x, Wg, bg, W1, b1, W2, b2):
    from concourse.bass_utils import run_bass_kernel_spmd

    B, S, d = x.shape
    T = B * S
    xt = x.reshape(T, d)

    tok_e, w_e = _route_host(xt, Wg, bg)
    counts = [len(t) for t in tok_e]
    C = max(128, ((max(counts) + 15) // 16) * 16)

    if C not in _NC_CACHE:
        _NC_CACHE[C] = _build_moe_nc(C)
    nc = _NC_CACHE[C]

    in_maps = _make_in_maps(xt, W1, b1, W2, b2, tok_e, counts, C)
    res = run_bass_kernel_spmd(nc, in_maps, core_ids=list(range(N_CORES)))

    out = np.zeros((T, d), dtype=np.float32)
    for e in range(NUM_EXPERTS):
        cnt = counts[e]
        if cnt:
            y_e = res.results[e]["y"]  # [d, C] bf16
            out[tok_e[e]] += y_e[:, :cnt].T.astype(np.float32) * w_e[e][:, None]
    return out.reshape(B, S, d)


# revision 17
# speedup vs baseline: 1.0073x; 1.0073x over previous
"""MoE FFN (8 experts, top-2) on 8 TRN2 NeuronCores — expert-parallel.

Strategy:
  - Host (inside kernel()): compute gate logits (fp64), top-2 selection +
    softmax weights exactly as the reference; gather each expert's tokens
    into a padded transposed buffer (bf16).
  - Device (SPMD, one expert per core): hT = relu(W1^T @ x + b1) then
    yT = W2^T @ hT + b2, bf16 matmuls with fp32 PSUM accumulation.
    All inputs are host-packed so every DMA reads >=6KB contiguous rows.
  - Host: out[token] += gate_weight * yT[:, col].T  (scatter-add combine).

Shapes (hardcoded from the problem):
  x: [4, 1024, 1024] f32, Wg: [1024, 8], bg: [8],
  W1: [8, 1024, 4096], b1: [8, 4096], W2: [8, 4096, 1024], b2: [8, 1024]
"""

import math

import ml_dtypes
import numpy as np

MODEL_DIM = 1024
DIM_FF = 4096
NUM_EXPERTS = 8
TOP_K = 2
N_CORES = 8

BF16 = ml_dtypes.bfloat16

_NC_CACHE: dict[int, object] = {}


def _ttile_split(C):
    n_tt = math.ceil(C / 512)
    ncols = [C // n_tt + (1 if i < C % n_tt else 0) for i in range(n_tt)]
    col0s = [sum(ncols[:i]) for i in range(n_tt)]
    return n_tt, ncols, col0s


def _build_moe_nc(C: int):
    """Build + compile the per-core Bass program for token capacity C.

    Inputs (per core, all host-packed for contiguous DMA):
      xs   [128, 8*C]  bf16 — token block tt at cols 8*col0[tt], dk-major inside
      w1   [128, 32768] bf16 — block (c,h) at col (c*2+h)*4096; inside, dk=4h+j
                               occupies cols j*1024..; covers W1[dk-rows, c-cols]
      w2   [128, 32768] bf16 — block g at col g*4096; fk=4g+j at j*1024..
      bias [128, 40]   f32  — cols 0:32 = b1 f-blocks, 32:40 = b2 d-blocks
    Output: y [1024, C] bf16 (= W2^T @ relu(W1^T @ xs + b1) + b2, transposed).
    """
    import concourse.mybir as mybir
    import concourse.tile as tile
    from concourse import bacc

    d, f = MODEL_DIM, DIM_FF
    DKS = d // 128   # 8
    FKS = f // 128   # 32
    n_tt, ncols, col0s = _ttile_split(C)

    # head = xs-tt0 + fm0..1 lhsT dup + bias(f32 bits as bf16 cols), ONE DMA
    HEAD_XS = 8 * ncols[0]
    HEAD_COLS = HEAD_XS + 2048 + 80

    nc = bacc.Bacc("TRN2", target_bir_lowering=False)
    head_d = nc.dram_tensor("head", [128, HEAD_COLS], mybir.dt.bfloat16,
                            kind="ExternalInput")
    xs_d = nc.dram_tensor("xs", [128, 8 * C], mybir.dt.bfloat16, kind="ExternalInput")
    w1_d = nc.dram_tensor("w1", [128, 32768], mybir.dt.bfloat16, kind="ExternalInput")
    w2_d = nc.dram_tensor("w2", [128, 32768], mybir.dt.bfloat16, kind="ExternalInput")
    y_d = nc.dram_tensor("y", [d, C], mybir.dt.bfloat16, kind="ExternalOutput")

    with tile.TileContext(nc) as tc:
        with (
            tc.tile_pool(name="pers", bufs=1) as pers,
            tc.tile_pool(name="xsp", bufs=2) as xsp,
            tc.tile_pool(name="hp", bufs=1) as hp,
            tc.tile_pool(name="yp", bufs=2) as yp,
            tc.tile_pool(name="psp", bufs=8, space="PSUM") as psp,
        ):
            headt = pers.tile([128, HEAD_COLS], mybir.dt.bfloat16,
                              tag="head", name="headt")
            nc.sync.dma_start(headt, head_d[:, :])

            # HAM warmup: dummy matmuls on a memset tile keep the PE busy
            # during the initial DMA wait so real matmuls start at 2.4 GHz.
            warm_sb = pers.tile([128, 512], mybir.dt.bfloat16,
                                tag="warm", name="warm_sb")
            nc.vector.memset(warm_sb, 0)
            warm_ps = psp.tile([128, 512], mybir.dt.float32, tag="ps",
                               name="warm_ps")
            N_WARM = 12
            for i in range(N_WARM):
                nc.tensor.matmul(
                    warm_ps, lhsT=warm_sb[:, :128], rhs=warm_sb,
                    start=(i == 0), stop=(i == N_WARM - 1),
                )

            xst = {0: headt[:, :HEAD_XS]}
            w1ht = headt[:, HEAD_XS:HEAD_XS + 2048]
            biast = headt[:, HEAD_XS + 2048:HEAD_XS + 2128].bitcast(
                mybir.dt.float32)

            def issue_xs(tt):
                ncol = ncols[tt]
                t = xsp.tile([128, 8 * ncol], mybir.dt.bfloat16,
                             tag="xs", name=f"xst_{tt}")
                nc.sync.dma_start(
                    t, xs_d[:, 8 * col0s[tt]:8 * col0s[tt] + 8 * ncol]
                )
                xst[tt] = t

            # w1 blocks (c, h): issued c-major to match phase A's fm-order reads
            w1t = {}
            for c in range(4):
                for h in range(2):
                    t = pers.tile([128, 4096], mybir.dt.bfloat16,
                                  tag=f"w1_{c}_{h}", name=f"w1t_{c}_{h}")
                    off = (c * 2 + h) * 4096
                    nc.sync.dma_start(t, w1_d[:, off:off + 4096])
                    w1t[c, h] = t
            if n_tt > 1:
                issue_xs(1)
            w2t = {}
            for g in range(8):
                t = pers.tile([128, 4096], mybir.dt.bfloat16,
                              tag=f"w2_{g}", name=f"w2t_{g}")
                nc.sync.dma_start(t, w2_d[:, g * 4096:(g + 1) * 4096])
                w2t[g] = t
            if n_tt > 2:
                issue_xs(2)

            for tt in range(n_tt):
                ncol, col0 = ncols[tt], col0s[tt]
                nxt = tt + 2
                if 3 <= nxt < n_tt:
                    issue_xs(nxt)

                # Phase A: hT[fm-block, cols] = relu(W1^T @ x + b1)
                hts = []
                for fm in range(FKS):
                    ps = psp.tile([128, ncol], mybir.dt.float32, tag="ps",
                                  name=f"ps1_{fm}_{tt}")
                    for dk in range(DKS):
                        if tt == 0 and fm < 2:
                            lhsT = w1ht[:, fm * 1024 + dk * 128:
                                        fm * 1024 + (dk + 1) * 128]
                        else:
                            lo = (dk % 4) * 1024 + (fm % 8) * 128
                            lhsT = w1t[fm // 8, dk // 4][:, lo:lo + 128]
                        nc.tensor.matmul(
                            ps,
                            lhsT=lhsT,
                            rhs=xst[tt][:, dk * ncol:(dk + 1) * ncol],
                            start=(dk == 0),
                            stop=(dk == DKS - 1),
                        )
                    ht = hp.tile([128, ncol], mybir.dt.bfloat16,
                                 tag=f"h_{fm}", name=f"ht_{fm}_{tt}")
                    # ht = max(ps + b1, 0) in one DVE op
                    nc.vector.tensor_scalar(
                        out=ht, in0=ps,
                        scalar1=biast[:, fm:fm + 1], scalar2=0.0,
                        op0=mybir.AluOpType.add, op1=mybir.AluOpType.max,
                    )
                    hts.append(ht)

                # Phase B: yT[dm-block, cols] = W2^T @ hT + b2
                for dm in range(DKS):
                    ps = psp.tile([128, ncol], mybir.dt.float32, tag="ps",
                                  name=f"ps2_{dm}_{tt}")
                    for fk in range(FKS):
                        lo = (fk % 4) * 1024 + dm * 128
                        nc.tensor.matmul(
                            ps,
                            lhsT=w2t[fk // 4][:, lo:lo + 128],
                            rhs=hts[fk],
                            start=(fk == 0),
                            stop=(fk == FKS - 1),
                        )
                    yo = yp.tile([128, ncol], mybir.dt.bfloat16, tag="yo",
                                 name=f"yo_{dm}_{tt}")
                    nc.vector.tensor_scalar_add(yo, ps, biast[:, 32 + dm:33 + dm])
                    nc.sync.dma_start(
                        y_d[dm * 128:(dm + 1) * 128, col0:col0 + ncol], yo
                    )

    nc.compile()
    return nc


def _pack_w1(W1e):
    """W1e [1024, 4096] f32 -> [128, 32768] bf16, block (c,h) layout."""
    W = W1e.astype(BF16)
    blocks = []
    for c in range(4):
        for h in range(2):
            blocks.append(np.concatenate(
                [W[(4 * h + j) * 128:(4 * h + j + 1) * 128,
                   c * 1024:(c + 1) * 1024] for j in range(4)], axis=1))
    return np.ascontiguousarray(np.concatenate(blocks, axis=1))


def _pack_w2(W2e):
    """W2e [4096, 1024] f32 -> [128, 32768] bf16, block g layout."""
    W = W2e.astype(BF16)
    blocks = []
    for g in range(8):
        blocks.append(np.concatenate(
            [W[(4 * g + j) * 128:(4 * g + j + 1) * 128, :] for j in range(4)],
            axis=1))
    return np.ascontiguousarray(np.concatenate(blocks, axis=1))


def _pack_w1h(W1e):
    """fm=0..1 lhsT blocks: [128, 2048] bf16, col fm*1024 + dk*128 -> W1 block."""
    W = W1e.astype(BF16)
    out = np.empty((128, 2048), dtype=BF16)
    for fm in range(2):
        for dk in range(8):
            out[:, fm * 1024 + dk * 128:fm * 1024 + (dk + 1) * 128] = \
                W[dk * 128:(dk + 1) * 128, fm * 128:(fm + 1) * 128]
    return np.ascontiguousarray(out)


def _pack_bias(b1e, b2e):
    bias = np.zeros((128, 40), np.float32)
    bias[:, :32] = b1e.astype(np.float32).reshape(32, 128).T
    bias[:, 32:40] = b2e.astype(np.float32).reshape(8, 128).T
    return np.ascontiguousarray(bias)


def _pack_xs(xT_bf16, C):
    """xT [1024, C] bf16 -> [128, 8*C] packed tt-major, dk inside."""
    _, ncols, col0s = _ttile_split(C)
    out = np.empty((128, 8 * C), dtype=BF16)
    for tt, (ncol, col0) in enumerate(zip(ncols, col0s)):
        for dk in range(8):
            out[:, 8 * col0 + dk * ncol: 8 * col0 + (dk + 1) * ncol] = \
                xT_bf16[dk * 128:(dk + 1) * 128, col0:col0 + ncol]
    return np.ascontiguousarray(out)


def _route_host(x, Wg, bg):
    """Reference-exact gate: fp64 logits, top-2 (ties -> lower index), softmax."""
    T = x.shape[0]
    logits = x.astype(np.float64) @ Wg.astype(np.float64) + bg.astype(np.float64)
    order = np.argsort(-logits, axis=1, kind="stable")[:, :TOP_K]  # [T, 2]
    vals = np.take_along_axis(logits, order, axis=1)
    vmax = vals.max(axis=1, keepdims=True)
    ev = np.exp(vals - vmax)
    w = (ev / ev.sum(axis=1, keepdims=True)).astype(np.float32)  # [T, 2]
    tok = np.repeat(np.arange(T), TOP_K)
    exp = order.ravel()
    wgt = w.ravel()
    tok_e, w_e = [], []
    for e in range(NUM_EXPERTS):
        m = exp == e
        tok_e.append(tok[m])
        w_e.append(wgt[m])
    return tok_e, w_e


def _make_in_maps(xt, W1, b1, W2, b2, tok_e, counts, C):
    _, ncols, _ = _ttile_split(C)
    in_maps = []
    for e in range(NUM_EXPERTS):
        xT = np.zeros((MODEL_DIM, C), dtype=BF16)
        cnt = counts[e]
        if cnt:
            xT[:, :cnt] = xt[tok_e[e]].astype(BF16).T
        xs_packed = _pack_xs(xT, C)
        head = np.ascontiguousarray(np.concatenate([
            xs_packed[:, :8 * ncols[0]],
            _pack_w1h(W1[e]),
            _pack_bias(b1[e], b2[e]).view(BF16),
        ], axis=1))
        in_maps.append({
            "head": head,
            "xs": xs_packed,
            "w1": _pack_w1(W1[e]),
            "w2": _pack_w2(W2[e]),
        })
    return in_maps


def kernel(x, Wg, bg, W1, b1, W2, b2):
    from concourse.bass_utils import run_bass_kernel_spmd

    B, S, d = x.shape
    T = B * S
    xt = x.reshape(T, d)

    tok_e, w_e = _route_host(xt, Wg, bg)
    counts = [len(t) for t in tok_e]
    C = max(128, ((max(counts) + 15) // 16) * 16)

    if C not in _NC_CACHE:
        _NC_CACHE[C] = _build_moe_nc(C)
    nc = _NC_CACHE[C]

    in_maps = _make_in_maps(xt, W1, b1, W2, b2, tok_e, counts, C)
    res = run_bass_kernel_spmd(nc, in_maps, core_ids=list(range(N_CORES)))

    out = np.zeros((T, d), dtype=np.float32)
    for e in range(NUM_EXPERTS):
        cnt = counts[e]
        if cnt:
            y_e = res.results[e]["y"]  # [d, C] bf16
            out[tok_e[e]] += y_e[:, :cnt].T.astype(np.float32) * w_e[e][:, None]
    return out.reshape(B, S, d)
